# revision 4
# baseline (speedup 1.0000x reference)
"""PointConv GNN message passing on 8 Trainium2 NeuronCores.

Strategy
--------
out_index is sorted, C_IN == 1, avg degree 16. The host reorders edges into a
dense (block, round, lane) grid: nodes are sorted by degree and grouped into
128-node blocks; round r of block b processes edge r of each of the block's
128 nodes (lane = node). Each node's edge list is padded to the block max
degree (~6% padding, zero-scale so pads contribute nothing).

This turns both gathers and the segment-sum into dense streaming work:
  - per-edge inputs (relative positions, scales) are staged by the host in
    exactly the SBUF layout the device consumes,
  - the segment-sum becomes a per-round accumulate into a [128 nodes, 64]
    SBUF accumulator where the scale is a per-partition scalar.

Device pipeline per 16-round macro (2048 edge slots):
  mm1: blockdiag(W1 x8) [24,128] @ posT [24,256] -> psum1 [128,256]   (f32r)
  celu1 = max(x, min(exp(x)-1, 0)):  Exp on ACT, min on GPSIMD, max on DVE
  mm2 (x2): t1 [128,128] @ blockdiag(W2 x8) [128,512] -> psum2 [128,512]
  celu2 likewise; then per round: acc = (v_r * s_r) + acc  (DVE stt)
Block end: PE-transpose acc, mm3 with W3 (4 blocks batched), +b3 on ACT, DMA.

Nodes are partitioned across cores (whole blocks dealt round-robin), so there
are no cross-core reductions at all. All 8 cores run one identical program.
"""

import numpy as np

# ---------------------------------------------------------------- constants
N_IN = 100000
N_OUT = 100000
E = 1600000
C_MID = 64
C_OUT = 64
POS_DIM = 3
N_CORES = 8
BLK = 128          # nodes per block (= lanes)
RPM = 16           # rounds per mm1 macro
SUB = 8            # rounds per mm2 subgroup
MM3_BATCH = 4      # blocks per mm3

_COMPILED = {}     # (structure key) -> (nc, sharded_fn, meta)


# =================================================================== host ==
def _schedule(deg):
    """Build the shared block/round structure and per-core node lists.

    Returns dict with:
      nodes   : [N_CORES, NB, BLK] int32 node ids (-1 pad)
      D       : [NB] int32 rounds per block (shared across cores)
      Roff    : [NB] int32 round offset of each block
      R_pad   : int, total rounds padded to RPM multiple
    """
    order = np.argsort(-deg, kind="stable").astype(np.int32)
    nbg = -(-N_OUT // BLK)                       # global blocks
    pad = nbg * BLK - N_OUT
    order_p = np.concatenate([order, np.full(pad, -1, np.int32)])
    gblocks = order_p.reshape(nbg, BLK)          # global block -> node ids
    gdeg = np.where(gblocks >= 0, deg[np.maximum(gblocks, 0)], 0)
    gmax = gdeg.max(axis=1)                      # per global block max degree

    NB = -(-nbg // N_CORES)
    nodes = np.full((N_CORES, NB, BLK), -1, np.int32)
    D = np.zeros(NB, np.int32)
    for k in range(NB):
        for c in range(N_CORES):
            g = k * N_CORES + c
            if g < nbg:
                nodes[c, k] = gblocks[g]
                D[k] = max(D[k], gmax[g])
    Roff = np.concatenate([[0], np.cumsum(D)]).astype(np.int32)
    R = int(Roff[-1])
    R_pad = -(-R // RPM) * RPM
    return dict(nodes=nodes, D=D, Roff=Roff, R=R, R_pad=R_pad, NB=NB)


def _edge_table(sch, deg, start, core):
    """[R_pad, BLK] int32 edge ids for one core (-1 = pad slot)."""
    nodes = sch["nodes"][core]          # [NB, BLK]
    D, Roff, R_pad = sch["D"], sch["Roff"], sch["R_pad"]
    R = sch["R"]
    blk_of_row = np.repeat(np.arange(len(D), dtype=np.int32), D)    # [R]
    r_in_blk = np.arange(R, dtype=np.int32) - Roff[blk_of_row]
    nd = nodes[blk_of_row]                                          # [R, BLK]
    dg = np.where(nd >= 0, deg[np.maximum(nd, 0)], 0)
    st = np.where(nd >= 0, start[np.maximum(nd, 0)], 0)
    eid = st + r_in_blk[:, None]
    valid = (nd >= 0) & (r_in_blk[:, None] < dg)
    eid = np.where(valid, eid, -1).astype(np.int32)
    out = np.full((R_pad, BLK), -1, np.int32)
    out[:R] = eid
    return out


def _core_arrays(sch, eids, pos_local, s_edge):
    """Device input arrays for one core."""
    R_pad = sch["R_pad"]
    U = R_pad // SUB
    idx = np.where(eids >= 0, eids, E)               # E -> zero pad row
    plp = np.concatenate([pos_local, np.zeros((1, POS_DIM), np.float32)])
    sp = np.concatenate([s_edge, np.zeros(1, np.float32)])
    pos_r = plp[idx]                                  # [R_pad, BLK, 3]
    # A_pos[3*rr + a, 128*u + lane] = pos_r[8u + rr, lane, a]
    A_pos = np.ascontiguousarray(
        pos_r.reshape(U, SUB, BLK, POS_DIM).transpose(1, 3, 0, 2)
        .reshape(SUB * POS_DIM, U * BLK))
    S_T = np.ascontiguousarray(sp[idx].T)             # [BLK, R_pad]
    return A_pos, S_T


def _weights_arrays(W1, W2, W3, b3):
    W1b = np.zeros((SUB * POS_DIM, BLK), np.float32)
    for r in range(SUB):
        W1b[3 * r:3 * r + 3, 16 * r:16 * r + 16] = W1
    W2b = np.zeros((BLK, SUB * C_MID), np.float32)
    for r in range(SUB):
        W2b[16 * r:16 * r + 16, 64 * r:64 * r + 64] = W2
    ident = np.eye(BLK, dtype=np.float32)
    return W1b, W2b, np.ascontiguousarray(W3.astype(np.float32)), \
        b3.reshape(C_OUT, 1).astype(np.float32), ident


def _prepare(x_in, pos_in, pos_out, in_index, out_index, W1, W2, W3, b3):
    i32 = np.asarray(in_index, np.int64).astype(np.int32)
    o32 = np.asarray(out_index, np.int64).astype(np.int32)
    x = np.asarray(x_in, np.float32).reshape(-1)
    deg = np.bincount(o32, minlength=N_OUT).astype(np.int32)
    start = np.concatenate([[0], np.cumsum(deg)[:-1]]).astype(np.int64)
    degc = np.maximum(deg, 1).astype(np.float32)
    s_edge = (x[i32] / degc[o32]).astype(np.float32)
    pos_local = (np.asarray(pos_in, np.float32)[i32]
                 - np.asarray(pos_out, np.float32)[o32])
    sch = _schedule(deg)
    per_core = []
    for c in range(N_CORES):
        eids = _edge_table(sch, deg, start, c)
        A_pos, S_T = _core_arrays(sch, eids, pos_local, s_edge)
        per_core.append((A_pos, S_T))
    wts = _weights_arrays(np.asarray(W1, np.float32), np.asarray(W2, np.float32),
                          np.asarray(W3, np.float32), np.asarray(b3, np.float32))
    return sch, per_core, wts


def _emulate_core(sch, A_pos, S_T, W1, W2, W3, b3):
    """Numpy emulation of the device program for one core (fp32 math)."""
    R_pad, NB = sch["R_pad"], sch["NB"]
    U = R_pad // SUB
    pos_r = (A_pos.reshape(SUB, POS_DIM, U, BLK).transpose(2, 0, 3, 1)
             .reshape(R_pad, BLK, POS_DIM))
    x1 = pos_r @ W1                                   # [R, BLK, 16]
    t1 = np.maximum(x1, np.minimum(np.exp(x1) - 1, 0))
    x2 = t1 @ W2                                      # [R, BLK, 64]
    v = np.maximum(x2, np.minimum(np.exp(x2) - 1, 0))
    w = v * S_T.T[:, :, None]                         # [R, BLK, 64]
    out = np.zeros((C_OUT, NB * BLK), np.float32)
    for k in range(NB):
        r0, r1 = sch["Roff"][k], sch["Roff"][k + 1]
        if r1 == r0:
            continue
        acc = w[r0:r1].sum(axis=0)                    # [BLK, 64]
        ob = acc @ W3 + b3.reshape(1, -1)             # [BLK, 64]
        out[:, k * BLK:(k + 1) * BLK] = ob.T
    return out


def _unshard(sch, outs, b3):
    """Assemble full [N_OUT, C_OUT] output from per-core [64, NB*BLK]."""
    full = np.broadcast_to(b3.reshape(1, -1), (N_OUT, C_OUT)).astype(np.float32).copy()
    for c in range(N_CORES):
        nodes = sch["nodes"][c]                       # [NB, BLK]
        cols = outs[c].reshape(C_OUT, sch["NB"], BLK)
        mask = (nodes >= 0) & (sch["D"] > 0)[:, None]
        full[nodes[mask]] = cols[:, mask].T
    return full


# ================================================================= device ==
def _build_program(sch):
    import concourse.bass as bass
    import concourse.mybir as mybir
    import concourse.tile as tile
    from concourse import bacc
    from contextlib import ExitStack

    F32 = mybir.dt.float32
    F32R = mybir.dt.float32r
    AF = mybir.ActivationFunctionType
    OP = mybir.AluOpType

    R_pad, NB, D, Roff = sch["R_pad"], sch["NB"], sch["D"], sch["Roff"]
    U = R_pad // SUB
    G = R_pad // RPM

    # round -> block id (or -1)
    blk_of = np.full(R_pad, -1, np.int32)
    for k in range(NB):
        blk_of[Roff[k]:Roff[k + 1]] = k

    nc = bacc.Bacc("TRN2", target_bir_lowering=False, debug=False)

    a_pos = nc.dram_tensor("A_pos", [SUB * POS_DIM, U * BLK], F32R,
                           kind="ExternalInput").ap()
    s_t = nc.dram_tensor("S_T", [BLK, R_pad], F32, kind="ExternalInput").ap()
    w1b = nc.dram_tensor("W1b", [SUB * POS_DIM, BLK], F32R,
                         kind="ExternalInput").ap()
    w2b = nc.dram_tensor("W2b", [BLK, SUB * C_MID], F32R,
                         kind="ExternalInput").ap()
    w3 = nc.dram_tensor("W3", [C_MID, C_OUT], F32R, kind="ExternalInput").ap()
    b3 = nc.dram_tensor("b3", [C_OUT, 1], F32, kind="ExternalInput").ap()
    ident = nc.dram_tensor("ident", [BLK, BLK], F32, kind="ExternalInput").ap()
    out_d = nc.dram_tensor("out", [C_OUT, NB * BLK], F32,
                           kind="ExternalOutput").ap()

    with tile.TileContext(nc) as tc, ExitStack() as ctx:
        const = ctx.enter_context(tc.tile_pool(name="const", bufs=1))
        posp = ctx.enter_context(tc.tile_pool(name="posp", bufs=4))
        p1 = ctx.enter_context(tc.tile_pool(name="p1", bufs=3))
        p2 = ctx.enter_context(tc.tile_pool(name="p2", bufs=3))
        accp = ctx.enter_context(tc.tile_pool(name="accp", bufs=4))
        aggp = ctx.enter_context(tc.tile_pool(name="aggp", bufs=2))
        outp = ctx.enter_context(tc.tile_pool(name="outp", bufs=2))
        ps1 = ctx.enter_context(tc.tile_pool(name="ps1", bufs=2, space="PSUM"))
        ps2 = ctx.enter_context(tc.tile_pool(name="ps2", bufs=3, space="PSUM"))
        pst = ctx.enter_context(tc.tile_pool(name="pst", bufs=1, space="PSUM"))
        ps3 = ctx.enter_context(tc.tile_pool(name="ps3", bufs=1, space="PSUM"))

        # --- constants ----------------------------------------------------
        w1b_sb = const.tile([SUB * POS_DIM, BLK], F32R)
        nc.sync.dma_start(w1b_sb[:], w1b[:])
        w2b_sb = const.tile([BLK, SUB * C_MID], F32R)
        nc.sync.dma_start(w2b_sb[:], w2b[:])
        w3_sb = const.tile([C_MID, C_OUT], F32R)
        nc.sync.dma_start(w3_sb[:], w3[:])
        b3_sb = const.tile([C_OUT, 1], F32)
        nc.sync.dma_start(b3_sb[:], b3[:])
        id_sb = const.tile([BLK, BLK], F32)
        nc.sync.dma_start(id_sb[:], ident[:])
        st_sb = const.tile([BLK, R_pad], F32)
        nc.sync.dma_start(st_sb[:], s_t[:])

        acc_of_blk = {}
        done = []            # finished blocks awaiting mm3

        def flush_mm3():
            agg4 = aggp.tile([C_MID, MM3_BATCH * BLK], F32R, tag="agg4")
            for j, (k, acc) in enumerate(done):
                p_t = pst.tile([C_MID, BLK], F32, tag="pt")
                nc.tensor.transpose(p_t[:], acc[:], id_sb[:])
                nc.scalar.activation(agg4[:, j * BLK:(j + 1) * BLK], p_t[:],
                                     AF.Copy)
            nbat = len(done)
            p3 = ps3.tile([C_OUT, MM3_BATCH * BLK], F32, tag="p3")
            nc.tensor.matmul(p3[:, :nbat * BLK], w3_sb[:],
                             agg4[:, :nbat * BLK], start=True, stop=True)
            ot = outp.tile([C_OUT, MM3_BATCH * BLK], F32, tag="ot")
            nc.scalar.activation(ot[:, :nbat * BLK], p3[:, :nbat * BLK],
                                 AF.Identity, bias=b3_sb[:])
            k0 = done[0][0]
            nc.sync.dma_start(out_d[:, k0 * BLK:(k0 + nbat) * BLK],
                              ot[:, :nbat * BLK])
            done.clear()

        for g in range(G):
            pos_t = posp.tile([SUB * POS_DIM, 2 * BLK], F32R, tag="pos")
            nc.sync.dma_start(pos_t[:], a_pos[:, g * 2 * BLK:(g + 1) * 2 * BLK])

            psum1 = ps1.tile([BLK, 2 * BLK], F32, tag="psum1")
            nc.tensor.matmul(psum1[:], w1b_sb[:], pos_t[:], start=True,
                             stop=True)
            e1 = p1.tile([BLK, 2 * BLK], F32, tag="e1")
            nc.scalar.activation(e1[:], psum1[:], AF.Exp)
            m1 = p1.tile([BLK, 2 * BLK], F32, tag="m1")
            nc.gpsimd.tensor_scalar(out=m1[:], in0=e1[:], scalar1=-1.0,
                                    scalar2=0.0, op0=OP.add, op1=OP.min)
            t1 = p1.tile([BLK, 2 * BLK], F32R, tag="t1")
            nc.vector.tensor_tensor(out=t1[:], in0=psum1[:], in1=m1[:],
                                    op=OP.max)

            for h in range(2):
                psum2 = ps2.tile([BLK, SUB * C_MID], F32, tag="psum2")
                nc.tensor.matmul(psum2[:], t1[:, h * BLK:(h + 1) * BLK],
                                 w2b_sb[:], start=True, stop=True)
                e2 = p2.tile([BLK, SUB * C_MID], F32, tag="e2")
                nc.scalar.activation(e2[:], psum2[:], AF.Exp)
                m2 = p2.tile([BLK, SUB * C_MID], F32, tag="m2")
                nc.gpsimd.tensor_scalar(out=m2[:], in0=e2[:], scalar1=-1.0,
                                        scalar2=0.0, op0=OP.add, op1=OP.min)
                v = p2.tile([BLK, SUB * C_MID], F32, tag="v")
                nc.vector.tensor_tensor(out=v[:], in0=psum2[:], in1=m2[:],
                                        op=OP.max)

                for r in range(SUB):
                    R = g * RPM + h * SUB + r
                    k = blk_of[R]
                    if k < 0:
                        continue
                    vs = v[:, r * C_MID:(r + 1) * C_MID]
                    sc = st_sb[:, R:R + 1]
                    if R == Roff[k]:
                        acc = accp.tile([BLK, C_MID], F32, tag="acc")
                        acc_of_blk[k] = acc
                        nc.vector.tensor_scalar(out=acc[:], in0=vs,
                                                scalar1=sc, scalar2=0.0,
                                                op0=OP.mult, op1=OP.add)
                    else:
                        acc = acc_of_blk[k]
                        nc.vector.scalar_tensor_tensor(out=acc[:], in0=vs,
                                                       scalar=sc, in1=acc[:],
                                                       op0=OP.mult, op1=OP.add)
                    if R == Roff[k] + D[k] - 1:
                        done.append((k, acc_of_blk.pop(k)))
                        if len(done) == MM3_BATCH:
                            flush_mm3()
        if done:
            flush_mm3()

    nc.compile()
    return nc


# ================================================================= runner ==
class _Runner:
    """Cached shard_map-jit wrapper around the bass_exec custom call.

    Mirrors concourse.bass2jax.run_bass_via_pjrt's multi-core branch, but
    keeps the jitted callable (and optionally device-resident inputs) so the
    kernel can be re-executed without re-tracing/re-compiling.
    """

    def __init__(self, nc):
        import jax
        import numpy as np
        from jax.sharding import Mesh, PartitionSpec, NamedSharding
        from jax.experimental.shard_map import shard_map
        import concourse.mybir as mybir
        from concourse.bass2jax import (_bass_exec_p, install_neuronx_cc_hook,
                                        partition_id_tensor)

        install_neuronx_cc_hook()
        self.nc = nc
        part_name = (nc.partition_id_tensor.name
                     if nc.partition_id_tensor is not None else None)
        in_names, out_names, out_avals, zero_outs = [], [], [], []
        for alloc in nc.m.functions[0].allocations:
            if not isinstance(alloc, mybir.MemoryLocationSet):
                continue
            name = alloc.memorylocations[0].name
            if alloc.kind == "ExternalInput":
                if name != part_name:
                    in_names.append(name)
            elif alloc.kind == "ExternalOutput":
                shape = tuple(alloc.tensor_shape)
                dtype = mybir.dt.np(alloc.dtype)
                out_names.append(name)
                out_avals.append(jax.core.ShapedArray(shape, dtype))
                zero_outs.append(np.zeros(shape, dtype))
        self.in_names = list(in_names)
        self.out_names = out_names
        self.out_avals = out_avals
        self.zero_outs = zero_outs
        all_in_names = list(in_names) + list(out_names)
        if part_name is not None:
            all_in_names.append(part_name)
        all_in_names = tuple(all_in_names)

        def _body(*args):
            operands = list(args)
            if part_name is not None:
                operands.append(partition_id_tensor())
            outs = _bass_exec_p.bind(
                *operands, out_avals=tuple(out_avals), in_names=all_in_names,
                out_names=tuple(out_names), lowering_input_output_aliases=(),
                sim_require_finite=True, sim_require_nnan=True, nc=nc)
            return tuple(outs)

        devices = jax.devices()[:N_CORES]
        self.mesh = Mesh(np.asarray(devices), ("core",))
        n_args = len(in_names) + len(zero_outs)
        in_specs = (PartitionSpec("core"),) * n_args
        out_specs = (PartitionSpec("core"),) * len(out_names)
        self.sharding = NamedSharding(self.mesh, PartitionSpec("core"))
        self.fn = jax.jit(shard_map(_body, mesh=self.mesh, in_specs=in_specs,
                                    out_specs=out_specs, check_rep=False),
                          keep_unused=True)

    def stage(self, in_maps):
        """Concatenate per-core inputs and place them on the devices."""
        import jax
        import numpy as np
        concat = [np.concatenate([np.asarray(m[n]) for m in in_maps], axis=0)
                  for n in self.in_names]
        concat += [np.concatenate([z] * N_CORES, axis=0)
                   for z in self.zero_outs]
        return [jax.device_put(a, self.sharding) for a in concat]

    def run(self, staged):
        import jax
        outs = self.fn(*staged)
        jax.block_until_ready(outs)
        return outs

    def results(self, outs):
        import numpy as np
        per_core = []
        for c in range(N_CORES):
            per_core.append({
                n: np.asarray(outs[i]).reshape(N_CORES, *self.out_avals[i].shape)[c]
                for i, n in enumerate(self.out_names)})
        return per_core


_RUNNER = None
_LAST = {}


def _run_on_device(nc, in_maps):
    global _RUNNER
    if _RUNNER is None or _RUNNER.nc is not nc:
        _RUNNER = _Runner(nc)
    staged = _RUNNER.stage(in_maps)
    outs = _RUNNER.run(staged)
    _LAST["runner"] = _RUNNER
    _LAST["staged"] = staged
    return [r["out"] for r in _RUNNER.results(outs)]


def kernel(x_in, pos_in, batch_in, pos_out, in_index, out_index,
           W1, W2, W3, b3):
    sch, per_core, wts = _prepare(x_in, pos_in, pos_out, in_index, out_index,
                                  W1, W2, W3, b3)
    W1b, W2b, W3a, b3a, ident = wts

    key = (sch["R_pad"], sch["NB"], tuple(sch["D"].tolist()))
    if key not in _COMPILED:
        _COMPILED.clear()
        _COMPILED[key] = _build_program(sch)
    nc = _COMPILED[key]

    in_maps = []
    for c in range(N_CORES):
        A_pos, S_T = per_core[c]
        in_maps.append({"A_pos": A_pos, "S_T": S_T, "W1b": W1b, "W2b": W2b,
                        "W3": W3a, "b3": b3a, "ident": ident})
    outs = _run_on_device(nc, in_maps)
    return _unshard(sch, outs, b3a)


# revision 31
# speedup vs baseline: 1.1046x; 1.1046x over previous
"""PointConv GNN message passing on 8 Trainium2 NeuronCores.

Strategy
--------
out_index is sorted, C_IN == 1, avg degree 16. The host reorders edges into a
dense (block, round, lane) grid: nodes are sorted by degree and grouped into
128-node blocks; round r of block b processes edge r of each of the block's
128 nodes (lane = node). Each node's edge list is padded to the block max
degree (~6% padding, zero-scale so pads contribute nothing).

This turns both gathers and the segment-sum into dense streaming work:
  - per-edge inputs (relative positions, scales) are staged by the host in
    exactly the SBUF layout the device consumes,
  - the segment-sum becomes a per-round accumulate into a [128 nodes, 64]
    SBUF accumulator where the scale is a per-partition scalar.

Device pipeline per 16-round macro (2048 edge slots):
  mm1: blockdiag(W1 x8) [24,128] @ posT [24,256] -> psum1 [128,256]   (f32r)
  celu1 = max(x, min(exp(x)-1, 0)):  Exp on ACT, min on GPSIMD, max on DVE
  mm2 (x2): t1 [128,128] @ blockdiag(W2 x8) [128,512] -> psum2 [128,512]
  celu2 likewise; then per round: acc = (v_r * s_r) + acc  (DVE stt)
Block end: PE-transpose acc, mm3 with W3 (4 blocks batched), +b3 on ACT, DMA.

Nodes are partitioned across cores (whole blocks dealt round-robin), so there
are no cross-core reductions at all. All 8 cores run one identical program.
"""

import numpy as np

# ---------------------------------------------------------------- constants
N_IN = 100000
N_OUT = 100000
E = 1600000
C_MID = 64
C_OUT = 64
POS_DIM = 3
N_CORES = 8
BLK = 128          # nodes per block (= lanes)
RPM = 32           # rounds per mm1 super-macro (pos tile)
SUB = 8            # rounds per mm2 matmul (blockdiag W2 x8)
MM2G = 16          # rounds per celu2 tile ([128, 1024] psum)
MM3_BATCH = 4      # blocks per mm3

_COMPILED = {}     # (structure key) -> (nc, sharded_fn, meta)
POOL_ACC_NUM, POOL_ACC_DEN = 5, 17  # fraction of blocks whose acc chain runs
                                     # on GPSIMD instead of DVE
PS1_BUFS, PS2_BUFS = 2, 2           # PSUM double-buffer depths (banks are
                                    # scarce: ps1*1 + ps2*2 + 2 <= 8)
PST_BUFS = 1
SHARE_PST = False                   # share one PSUM pool between transpose+mm3
P1_BUFS, P2_BUFS = 4, 4


# =================================================================== host ==
def _schedule(deg):
    """Build the shared block/round structure and per-core node lists.

    Returns dict with:
      nodes   : [N_CORES, NB, BLK] int32 node ids (-1 pad)
      D       : [NB] int32 rounds per block (shared across cores)
      Roff    : [NB] int32 round offset of each block
      R_pad   : int, total rounds padded to RPM multiple
    """
    order = np.argsort(-deg, kind="stable").astype(np.int32)
    nbg = -(-N_OUT // BLK)                       # global blocks
    pad = nbg * BLK - N_OUT
    order_p = np.concatenate([order, np.full(pad, -1, np.int32)])
    gblocks = order_p.reshape(nbg, BLK)          # global block -> node ids
    gdeg = np.where(gblocks >= 0, deg[np.maximum(gblocks, 0)], 0)
    gmax = gdeg.max(axis=1)                      # per global block max degree

    NB = -(-nbg // N_CORES)
    nodes = np.full((N_CORES, NB, BLK), -1, np.int32)
    D = np.zeros(NB, np.int32)
    for k in range(NB):
        for c in range(N_CORES):
            g = k * N_CORES + c
            if g < nbg:
                nodes[c, k] = gblocks[g]
                D[k] = max(D[k], gmax[g])
    Roff = np.concatenate([[0], np.cumsum(D)]).astype(np.int32)
    R = int(Roff[-1])
    R_pad = -(-R // RPM) * RPM
    return dict(nodes=nodes, D=D, Roff=Roff, R=R, R_pad=R_pad, NB=NB)


def _edge_table(sch, deg, start, core):
    """[R_pad, BLK] int32 edge ids for one core (-1 = pad slot)."""
    nodes = sch["nodes"][core]          # [NB, BLK]
    D, Roff, R_pad = sch["D"], sch["Roff"], sch["R_pad"]
    R = sch["R"]
    blk_of_row = np.repeat(np.arange(len(D), dtype=np.int32), D)    # [R]
    r_in_blk = np.arange(R, dtype=np.int32) - Roff[blk_of_row]
    nd = nodes[blk_of_row]                                          # [R, BLK]
    dg = np.where(nd >= 0, deg[np.maximum(nd, 0)], 0)
    st = np.where(nd >= 0, start[np.maximum(nd, 0)], 0)
    eid = st + r_in_blk[:, None]
    valid = (nd >= 0) & (r_in_blk[:, None] < dg)
    eid = np.where(valid, eid, -1).astype(np.int32)
    out = np.full((R_pad, BLK), -1, np.int32)
    out[:R] = eid
    return out


def _core_arrays(sch, eids, pos_local, s_edge):
    """Device input arrays for one core."""
    R_pad = sch["R_pad"]
    U = R_pad // SUB
    idx = np.where(eids >= 0, eids, E)               # E -> zero pad row
    plp = np.concatenate([pos_local, np.zeros((1, POS_DIM), np.float32)])
    sp = np.concatenate([s_edge, np.zeros(1, np.float32)])
    pos_r = plp[idx]                                  # [R_pad, BLK, 3]
    # A_pos[3*rr + a, 128*u + lane] = pos_r[8u + rr, lane, a]
    A_pos = np.ascontiguousarray(
        pos_r.reshape(U, SUB, BLK, POS_DIM).transpose(1, 3, 0, 2)
        .reshape(SUB * POS_DIM, U * BLK))
    S_T = np.ascontiguousarray(sp[idx].T)             # [BLK, R_pad]
    return A_pos, S_T


def _weights_arrays(W1, W2, W3, b3):
    W1b = np.zeros((SUB * POS_DIM, BLK), np.float32)
    for r in range(SUB):
        W1b[3 * r:3 * r + 3, 16 * r:16 * r + 16] = W1
    W2b = np.zeros((BLK, SUB * C_MID), np.float32)
    for r in range(SUB):
        W2b[16 * r:16 * r + 16, 64 * r:64 * r + 64] = W2
    ident = np.eye(BLK, dtype=np.float32)
    return W1b, W2b, np.ascontiguousarray(W3.astype(np.float32)), \
        b3.reshape(C_OUT, 1).astype(np.float32), ident


def _prepare(x_in, pos_in, pos_out, in_index, out_index, W1, W2, W3, b3):
    i32 = np.asarray(in_index, np.int64).astype(np.int32)
    o32 = np.asarray(out_index, np.int64).astype(np.int32)
    x = np.asarray(x_in, np.float32).reshape(-1)
    deg = np.bincount(o32, minlength=N_OUT).astype(np.int32)
    start = np.concatenate([[0], np.cumsum(deg)[:-1]]).astype(np.int64)
    degc = np.maximum(deg, 1).astype(np.float32)
    s_edge = (x[i32] / degc[o32]).astype(np.float32)
    pos_local = (np.asarray(pos_in, np.float32)[i32]
                 - np.asarray(pos_out, np.float32)[o32])
    sch = _schedule(deg)
    per_core = []
    for c in range(N_CORES):
        eids = _edge_table(sch, deg, start, c)
        A_pos, S_T = _core_arrays(sch, eids, pos_local, s_edge)
        per_core.append((A_pos, S_T))
    wts = _weights_arrays(np.asarray(W1, np.float32), np.asarray(W2, np.float32),
                          np.asarray(W3, np.float32), np.asarray(b3, np.float32))
    return sch, per_core, wts


def _emulate_core(sch, A_pos, S_T, W1, W2, W3, b3):
    """Numpy emulation of the device program for one core (fp32 math)."""
    R_pad, NB = sch["R_pad"], sch["NB"]
    U = R_pad // SUB
    pos_r = (A_pos.reshape(SUB, POS_DIM, U, BLK).transpose(2, 0, 3, 1)
             .reshape(R_pad, BLK, POS_DIM))
    x1 = pos_r @ W1                                   # [R, BLK, 16]
    t1 = np.maximum(x1, np.minimum(np.exp(x1) - 1, 0))
    x2 = t1 @ W2                                      # [R, BLK, 64]
    v = np.maximum(x2, np.minimum(np.exp(x2) - 1, 0))
    w = v * S_T.T[:, :, None]                         # [R, BLK, 64]
    out = np.zeros((C_OUT, NB * BLK), np.float32)
    for k in range(NB):
        r0, r1 = sch["Roff"][k], sch["Roff"][k + 1]
        if r1 == r0:
            continue
        acc = w[r0:r1].sum(axis=0)                    # [BLK, 64]
        ob = acc @ W3 + b3.reshape(1, -1)             # [BLK, 64]
        out[:, k * BLK:(k + 1) * BLK] = ob.T
    return out


def _unshard(sch, outs, b3):
    """Assemble full [N_OUT, C_OUT] output from per-core [64, NB*BLK]."""
    full = np.broadcast_to(b3.reshape(1, -1), (N_OUT, C_OUT)).astype(np.float32).copy()
    for c in range(N_CORES):
        nodes = sch["nodes"][c]                       # [NB, BLK]
        cols = outs[c].reshape(C_OUT, sch["NB"], BLK)
        mask = (nodes >= 0) & (sch["D"] > 0)[:, None]
        full[nodes[mask]] = cols[:, mask].T
    return full


# ================================================================= device ==
_CELU_OP = None


def _register_celu_op():
    """Register a fused custom DVE op: out = max(in1, min(in0 + imm2, 0)).

    With in0 = exp(x), in1 = x, imm2 = -1 this computes celu(x) in a single
    DVE instruction (celu(x) = max(x, min(e^x - 1, 0))), replacing an ACT/
    GPSIMD pass plus a DVE pass with one DVE pass.
    """
    global _CELU_OP
    if _CELU_OP is not None:
        return _CELU_OP
    import numpy as np
    import concourse.dve_ops as dve_ops
    from concourse.dve_spec import (Spec, Src0, Src1, C2, Zero, lower,
                                    maxx, minn, _has_src1)
    from concourse.dve_uop import DveOpSpec

    name = "CELU_FUSED_PC"
    body = maxx(Src1, minn(Src0 + C2, Zero))
    spec = Spec(
        body=body,
        reference=lambda in0, in1, s0, s1, imm2: np.maximum(
            in1.astype(np.float32),
            np.minimum(in0.astype(np.float32) + imm2, 0.0)).astype(np.float32),
    )
    row = dve_ops._CUSTOM_DVE_ROW_BASE + len(dve_ops.OPS)
    assert row < 0x20
    shas = {}
    for ver in ("v3", "v4"):
        ds = DveOpSpec(name=name, opcode=row, uops=lower(spec, ver=ver),
                       rd1_en=_has_src1(spec))
        shas[ver] = ds.sha(ver)
    op = dve_ops.DveOp(name, spec, subdim=False, uops_sha=shas)
    dve_ops.OPS.append(op)
    dve_ops._SUB_OPCODE_FOR_NAME[name] = row
    dve_ops.CUSTOM_DVE_SPECS[name] = spec
    _CELU_OP = op
    return op


def _build_program(sch):
    import concourse.bass as bass
    import concourse.mybir as mybir
    import concourse.tile as tile
    from concourse import bacc
    from contextlib import ExitStack

    F32 = mybir.dt.float32
    F32R = mybir.dt.float32r
    AF = mybir.ActivationFunctionType
    OP = mybir.AluOpType

    R_pad, NB, D, Roff = sch["R_pad"], sch["NB"], sch["D"], sch["Roff"]
    U = R_pad // SUB
    G = R_pad // RPM

    # round -> block id (or -1)
    blk_of = np.full(R_pad, -1, np.int32)
    for k in range(NB):
        blk_of[Roff[k]:Roff[k + 1]] = k

    celu_op = _register_celu_op()

    nc = bacc.Bacc("TRN2", target_bir_lowering=False, debug=False)

    a_pos = nc.dram_tensor("A_pos", [SUB * POS_DIM, U * BLK], F32R,
                           kind="ExternalInput").ap()
    s_t = nc.dram_tensor("S_T", [BLK, R_pad], F32, kind="ExternalInput").ap()
    w1b = nc.dram_tensor("W1b", [SUB * POS_DIM, BLK], F32R,
                         kind="ExternalInput").ap()
    w2b = nc.dram_tensor("W2b", [BLK, SUB * C_MID], F32R,
                         kind="ExternalInput").ap()
    w3 = nc.dram_tensor("W3", [C_MID, C_OUT], F32R, kind="ExternalInput").ap()
    b3 = nc.dram_tensor("b3", [C_OUT, 1], F32, kind="ExternalInput").ap()
    ident = nc.dram_tensor("ident", [BLK, BLK], F32, kind="ExternalInput").ap()
    out_d = nc.dram_tensor("out", [C_OUT, NB * BLK], F32,
                           kind="ExternalOutput").ap()

    with tile.TileContext(nc) as tc, ExitStack() as ctx:
        const = ctx.enter_context(tc.tile_pool(name="const", bufs=1))
        posp = ctx.enter_context(tc.tile_pool(name="posp", bufs=4))
        p1 = ctx.enter_context(tc.tile_pool(name="p1", bufs=P1_BUFS))
        p2 = ctx.enter_context(tc.tile_pool(name="p2", bufs=P2_BUFS))
        accp = ctx.enter_context(tc.tile_pool(name="accp", bufs=6))
        aggp = ctx.enter_context(tc.tile_pool(name="aggp", bufs=2))
        outp = ctx.enter_context(tc.tile_pool(name="outp", bufs=2))
        ps1 = ctx.enter_context(tc.tile_pool(name="ps1", bufs=PS1_BUFS,
                                             space="PSUM"))
        ps2 = ctx.enter_context(tc.tile_pool(name="ps2", bufs=PS2_BUFS,
                                             space="PSUM"))
        pst = ctx.enter_context(tc.tile_pool(name="pst", bufs=PST_BUFS,
                                             space="PSUM"))
        ps3 = pst if SHARE_PST else ctx.enter_context(
            tc.tile_pool(name="ps3", bufs=1, space="PSUM"))

        # --- constants ----------------------------------------------------
        w1b_sb = const.tile([SUB * POS_DIM, BLK], F32R)
        nc.sync.dma_start(w1b_sb[:], w1b[:])
        w2b_sb = const.tile([BLK, SUB * C_MID], F32R)
        nc.sync.dma_start(w2b_sb[:], w2b[:])
        w3_sb = const.tile([C_MID, C_OUT], F32R)
        nc.sync.dma_start(w3_sb[:], w3[:])
        b3_sb = const.tile([C_OUT, 1], F32)
        nc.sync.dma_start(b3_sb[:], b3[:])
        id_sb = const.tile([BLK, BLK], F32)
        nc.sync.dma_start(id_sb[:], ident[:])
        st_sb = const.tile([BLK, R_pad], F32)
        nc.sync.dma_start(st_sb[:], s_t[:])

        acc_of_blk = {}
        done = []            # finished blocks awaiting mm3

        def flush_mm3():
            nbat = len(done)
            p_t = pst.tile([C_MID, MM3_BATCH * BLK], F32,
                           tag="blkend" if SHARE_PST else "pt")
            for j, (k, acc) in enumerate(done):
                nc.tensor.transpose(p_t[:, j * BLK:(j + 1) * BLK], acc[:],
                                    id_sb[:])
            agg4 = aggp.tile([C_MID, MM3_BATCH * BLK], F32R, tag="agg4")
            nc.scalar.activation(agg4[:, :nbat * BLK], p_t[:, :nbat * BLK],
                                 AF.Copy)
            p3 = ps3.tile([C_OUT, MM3_BATCH * BLK], F32,
                          tag="blkend" if SHARE_PST else "p3")
            nc.tensor.matmul(p3[:, :nbat * BLK], w3_sb[:],
                             agg4[:, :nbat * BLK], start=True, stop=True)
            ot = outp.tile([C_OUT, MM3_BATCH * BLK], F32, tag="ot")
            nc.scalar.activation(ot[:, :nbat * BLK], p3[:, :nbat * BLK],
                                 AF.Identity, bias=b3_sb[:])
            k0 = done[0][0]
            nc.sync.dma_start(out_d[:, k0 * BLK:(k0 + nbat) * BLK],
                              ot[:, :nbat * BLK])
            done.clear()

        for g in range(G):
            pos_t = posp.tile([SUB * POS_DIM, 4 * BLK], F32R, tag="pos")
            nc.sync.dma_start(pos_t[:], a_pos[:, g * 4 * BLK:(g + 1) * 4 * BLK])

            psum1 = ps1.tile([BLK, 4 * BLK], F32, tag="psum1")
            nc.tensor.matmul(psum1[:], w1b_sb[:], pos_t[:], start=True,
                             stop=True)
            e1 = p1.tile([BLK, 4 * BLK], F32, tag="e1")
            nc.scalar.activation(e1[:], psum1[:], AF.Exp)
            # fused custom DVE op: t1 = max(x, min(e^x - 1, 0)) = celu(x)
            t1 = p1.tile([BLK, 4 * BLK], F32R, tag="t1")
            nc.vector._custom_dve(celu_op, out=t1[:], in0=e1[:],
                                  in1=psum1[:], imm2=-1.0)

            for h in range(RPM // MM2G):
                # two 8-round mm2's land in one [128, 1024] psum tile so the
                # celu2 passes amortize their fixed overheads
                psum2 = ps2.tile([BLK, MM2G * C_MID], F32, tag="psum2")
                for hh in range(MM2G // SUB):
                    u = (MM2G // SUB) * h + hh
                    nc.tensor.matmul(
                        psum2[:, hh * SUB * C_MID:(hh + 1) * SUB * C_MID],
                        t1[:, u * BLK:(u + 1) * BLK],
                        w2b_sb[:], start=True, stop=True)
                e2 = p2.tile([BLK, MM2G * C_MID], F32, tag="e2")
                nc.scalar.activation(e2[:], psum2[:], AF.Exp)
                v = p2.tile([BLK, MM2G * C_MID], F32, tag="v")
                nc.vector._custom_dve(celu_op, out=v[:], in0=e2[:],
                                      in1=psum2[:], imm2=-1.0)

                for r in range(MM2G):
                    R = g * RPM + h * MM2G + r
                    k = blk_of[R]
                    if k < 0:
                        continue
                    vs = v[:, r * C_MID:(r + 1) * C_MID]
                    sc = st_sb[:, R:R + 1]
                    on_pool = (k * POOL_ACC_NUM) % POOL_ACC_DEN < POOL_ACC_NUM
                    if R == Roff[k]:
                        acc = accp.tile([BLK, C_MID], F32, tag="acc")
                        acc_of_blk[k] = acc
                        eng = nc.gpsimd if on_pool else nc.vector
                        eng.tensor_scalar(out=acc[:], in0=vs, scalar1=sc,
                                          scalar2=0.0, op0=OP.mult, op1=OP.add)
                    elif on_pool:
                        # walrus rejects scalar_tensor_tensor on Pool; use a
                        # scale-into-temp + add pair instead
                        acc = acc_of_blk[k]
                        w = accp.tile([BLK, C_MID], F32, tag="accw")
                        nc.gpsimd.tensor_scalar(out=w[:], in0=vs, scalar1=sc,
                                                scalar2=0.0, op0=OP.mult,
                                                op1=OP.add)
                        nc.gpsimd.tensor_add(acc[:], acc[:], w[:])
                    else:
                        acc = acc_of_blk[k]
                        nc.vector.scalar_tensor_tensor(out=acc[:], in0=vs,
                                                       scalar=sc, in1=acc[:],
                                                       op0=OP.mult, op1=OP.add)
                    if R == Roff[k] + D[k] - 1:
                        done.append((k, acc_of_blk.pop(k)))
                        if len(done) == MM3_BATCH:
                            flush_mm3()
        if done:
            flush_mm3()

    nc.compile()
    return nc


# ================================================================= runner ==
class _Runner:
    """Cached shard_map-jit wrapper around the bass_exec custom call.

    Mirrors concourse.bass2jax.run_bass_via_pjrt's multi-core branch, but
    keeps the jitted callable (and optionally device-resident inputs) so the
    kernel can be re-executed without re-tracing/re-compiling.
    """

    def __init__(self, nc):
        import jax
        import numpy as np
        from jax.sharding import Mesh, PartitionSpec, NamedSharding
        from jax.experimental.shard_map import shard_map
        import concourse.mybir as mybir
        from concourse.bass2jax import (_bass_exec_p, install_neuronx_cc_hook,
                                        partition_id_tensor)

        install_neuronx_cc_hook()
        self.nc = nc
        part_name = (nc.partition_id_tensor.name
                     if nc.partition_id_tensor is not None else None)
        in_names, out_names, out_avals, zero_outs = [], [], [], []
        for alloc in nc.m.functions[0].allocations:
            if not isinstance(alloc, mybir.MemoryLocationSet):
                continue
            name = alloc.memorylocations[0].name
            if alloc.kind == "ExternalInput":
                if name != part_name:
                    in_names.append(name)
            elif alloc.kind == "ExternalOutput":
                shape = tuple(alloc.tensor_shape)
                dtype = mybir.dt.np(alloc.dtype)
                out_names.append(name)
                out_avals.append(jax.core.ShapedArray(shape, dtype))
                zero_outs.append(np.zeros(shape, dtype))
        self.in_names = list(in_names)
        self.out_names = out_names
        self.out_avals = out_avals
        self.zero_outs = zero_outs
        all_in_names = list(in_names) + list(out_names)
        if part_name is not None:
            all_in_names.append(part_name)
        all_in_names = tuple(all_in_names)

        def _body(*args):
            operands = list(args)
            if part_name is not None:
                operands.append(partition_id_tensor())
            outs = _bass_exec_p.bind(
                *operands, out_avals=tuple(out_avals), in_names=all_in_names,
                out_names=tuple(out_names), lowering_input_output_aliases=(),
                sim_require_finite=True, sim_require_nnan=True, nc=nc)
            return tuple(outs)

        devices = jax.devices()[:N_CORES]
        self.mesh = Mesh(np.asarray(devices), ("core",))
        n_args = len(in_names) + len(zero_outs)
        in_specs = (PartitionSpec("core"),) * n_args
        out_specs = (PartitionSpec("core"),) * len(out_names)
        self.sharding = NamedSharding(self.mesh, PartitionSpec("core"))
        self.fn = jax.jit(shard_map(_body, mesh=self.mesh, in_specs=in_specs,
                                    out_specs=out_specs, check_rep=False),
                          keep_unused=True)

    def stage(self, in_maps):
        """Concatenate per-core inputs and place them on the devices."""
        import jax
        import numpy as np
        concat = [np.concatenate([np.asarray(m[n]) for m in in_maps], axis=0)
                  for n in self.in_names]
        concat += [np.concatenate([z] * N_CORES, axis=0)
                   for z in self.zero_outs]
        return [jax.device_put(a, self.sharding) for a in concat]

    def run(self, staged):
        import jax
        outs = self.fn(*staged)
        jax.block_until_ready(outs)
        return outs

    def results(self, outs):
        import numpy as np
        per_core = []
        for c in range(N_CORES):
            per_core.append({
                n: np.asarray(outs[i]).reshape(N_CORES, *self.out_avals[i].shape)[c]
                for i, n in enumerate(self.out_names)})
        return per_core


_RUNNER = None
_LAST = {}


def _run_on_device(nc, in_maps):
    global _RUNNER
    if _RUNNER is None or _RUNNER.nc is not nc:
        _RUNNER = _Runner(nc)
    staged = _RUNNER.stage(in_maps)
    outs = _RUNNER.run(staged)
    _LAST["runner"] = _RUNNER
    _LAST["staged"] = staged
    return [r["out"] for r in _RUNNER.results(outs)]


def kernel(x_in, pos_in, batch_in, pos_out, in_index, out_index,
           W1, W2, W3, b3):
    sch, per_core, wts = _prepare(x_in, pos_in, pos_out, in_index, out_index,
                                  W1, W2, W3, b3)
    W1b, W2b, W3a, b3a, ident = wts

    key = (sch["R_pad"], sch["NB"], tuple(sch["D"].tolist()))
    if key not in _COMPILED:
        _COMPILED.clear()
        _COMPILED[key] = _build_program(sch)
    nc = _COMPILED[key]

    in_maps = []
    for c in range(N_CORES):
        A_pos, S_T = per_core[c]
        in_maps.append({"A_pos": A_pos, "S_T": S_T, "W1b": W1b, "W2b": W2b,
                        "W3": W3a, "b3": b3a, "ident": ident})
    outs = _run_on_device(nc, in_maps)
    return _unshard(sch, outs, b3a)


# revision 34
# speedup vs baseline: 290.9665x; 263.4122x over previous
"""PointConv GNN message passing on 8 Trainium2 NeuronCores.

Strategy
--------
out_index is sorted, C_IN == 1, avg degree 16. The host reorders edges into a
dense (block, round, lane) grid: nodes are sorted by degree and grouped into
128-node blocks; round r of block b processes edge r of each of the block's
128 nodes (lane = node). Each node's edge list is padded to the block max
degree (~6% padding, zero-scale so pads contribute nothing).

This turns both gathers and the segment-sum into dense streaming work:
  - per-edge inputs (relative positions, scales) are staged by the host in
    exactly the SBUF layout the device consumes,
  - the segment-sum becomes a per-round accumulate into a [128 nodes, 64]
    SBUF accumulator where the scale is a per-partition scalar.

Device pipeline per 16-round macro (2048 edge slots):
  mm1: blockdiag(W1 x8) [24,128] @ posT [24,256] -> psum1 [128,256]   (f32r)
  celu1 = max(x, min(exp(x)-1, 0)):  Exp on ACT, min on GPSIMD, max on DVE
  mm2 (x2): t1 [128,128] @ blockdiag(W2 x8) [128,512] -> psum2 [128,512]
  celu2 likewise; then per round: acc = (v_r * s_r) + acc  (DVE stt)
Block end: PE-transpose acc, mm3 with W3 (4 blocks batched), +b3 on ACT, DMA.

Nodes are partitioned across cores (whole blocks dealt round-robin), so there
are no cross-core reductions at all. All 8 cores run one identical program.
"""

import numpy as np

# ---------------------------------------------------------------- constants
N_IN = 100000
N_OUT = 100000
E = 1600000
C_MID = 64
C_OUT = 64
POS_DIM = 3
N_CORES = 8
BLK = 128          # nodes per block (= lanes)
RPM = 32           # rounds per mm1 super-macro (pos tile)
SUB = 8            # rounds per mm2 matmul (blockdiag W2 x8)
MM2G = 16          # rounds per celu2 tile ([128, 1024] psum)
MM3_BATCH = 4      # blocks per mm3

_COMPILED = {}     # (structure key) -> (nc, sharded_fn, meta)
POOL_ACC_NUM, POOL_ACC_DEN = 5, 17  # fraction of blocks whose acc chain runs
                                     # on GPSIMD instead of DVE
PS1_BUFS, PS2_BUFS = 2, 2           # PSUM double-buffer depths (banks are
                                    # scarce: ps1*1 + ps2*2 + 2 <= 8)
PST_BUFS = 1
SHARE_PST = False                   # share one PSUM pool between transpose+mm3
P1_BUFS, P2_BUFS = 4, 4


# =================================================================== host ==
def _schedule(deg):
    """Build the shared block/round structure and per-core node lists.

    Returns dict with:
      nodes   : [N_CORES, NB, BLK] int32 node ids (-1 pad)
      D       : [NB] int32 rounds per block (shared across cores)
      Roff    : [NB] int32 round offset of each block
      R_pad   : int, total rounds padded to RPM multiple
    """
    order = np.argsort(-deg, kind="stable").astype(np.int32)
    nbg = -(-N_OUT // BLK)                       # global blocks
    pad = nbg * BLK - N_OUT
    order_p = np.concatenate([order, np.full(pad, -1, np.int32)])
    gblocks = order_p.reshape(nbg, BLK)          # global block -> node ids
    gdeg = np.where(gblocks >= 0, deg[np.maximum(gblocks, 0)], 0)
    gmax = gdeg.max(axis=1)                      # per global block max degree

    NB = -(-nbg // N_CORES)
    nodes = np.full((N_CORES, NB, BLK), -1, np.int32)
    D = np.zeros(NB, np.int32)
    for k in range(NB):
        for c in range(N_CORES):
            g = k * N_CORES + c
            if g < nbg:
                nodes[c, k] = gblocks[g]
                D[k] = max(D[k], gmax[g])
    Roff = np.concatenate([[0], np.cumsum(D)]).astype(np.int32)
    R = int(Roff[-1])
    R_pad = -(-R // RPM) * RPM
    return dict(nodes=nodes, D=D, Roff=Roff, R=R, R_pad=R_pad, NB=NB)


def _edge_table(sch, deg, start, core):
    """[R_pad, BLK] int32 edge ids for one core (-1 = pad slot)."""
    nodes = sch["nodes"][core]          # [NB, BLK]
    D, Roff, R_pad = sch["D"], sch["Roff"], sch["R_pad"]
    R = sch["R"]
    blk_of_row = np.repeat(np.arange(len(D), dtype=np.int32), D)    # [R]
    r_in_blk = np.arange(R, dtype=np.int32) - Roff[blk_of_row]
    nd = nodes[blk_of_row]                                          # [R, BLK]
    dg = np.where(nd >= 0, deg[np.maximum(nd, 0)], 0)
    st = np.where(nd >= 0, start[np.maximum(nd, 0)], 0)
    eid = st + r_in_blk[:, None]
    valid = (nd >= 0) & (r_in_blk[:, None] < dg)
    eid = np.where(valid, eid, -1).astype(np.int32)
    out = np.full((R_pad, BLK), -1, np.int32)
    out[:R] = eid
    return out


def _core_arrays(sch, eids, pos_local, s_edge):
    """Device input arrays for one core."""
    R_pad = sch["R_pad"]
    U = R_pad // SUB
    idx = np.where(eids >= 0, eids, E)               # E -> zero pad row
    plp = np.concatenate([pos_local, np.zeros((1, POS_DIM), np.float32)])
    sp = np.concatenate([s_edge, np.zeros(1, np.float32)])
    pos_r = plp[idx]                                  # [R_pad, BLK, 3]
    # A_pos[3*rr + a, 128*u + lane] = pos_r[8u + rr, lane, a]
    A_pos = np.ascontiguousarray(
        pos_r.reshape(U, SUB, BLK, POS_DIM).transpose(1, 3, 0, 2)
        .reshape(SUB * POS_DIM, U * BLK))
    S_T = np.ascontiguousarray(sp[idx].T)             # [BLK, R_pad]
    return A_pos, S_T


def _weights_arrays(W1, W2, W3, b3):
    W1b = np.zeros((SUB * POS_DIM, BLK), np.float32)
    for r in range(SUB):
        W1b[3 * r:3 * r + 3, 16 * r:16 * r + 16] = W1
    W2b = np.zeros((BLK, SUB * C_MID), np.float32)
    for r in range(SUB):
        W2b[16 * r:16 * r + 16, 64 * r:64 * r + 64] = W2
    ident = np.eye(BLK, dtype=np.float32)
    return W1b, W2b, np.ascontiguousarray(W3.astype(np.float32)), \
        b3.reshape(C_OUT, 1).astype(np.float32), ident


def _prepare(x_in, pos_in, pos_out, in_index, out_index, W1, W2, W3, b3):
    i32 = np.asarray(in_index, np.int64).astype(np.int32)
    o32 = np.asarray(out_index, np.int64).astype(np.int32)
    x = np.asarray(x_in, np.float32).reshape(-1)
    deg = np.bincount(o32, minlength=N_OUT).astype(np.int32)
    start = np.concatenate([[0], np.cumsum(deg)[:-1]]).astype(np.int64)
    degc = np.maximum(deg, 1).astype(np.float32)
    s_edge = (x[i32] / degc[o32]).astype(np.float32)
    pos_local = (np.asarray(pos_in, np.float32)[i32]
                 - np.asarray(pos_out, np.float32)[o32])
    sch = _schedule(deg)
    per_core = []
    for c in range(N_CORES):
        eids = _edge_table(sch, deg, start, c)
        A_pos, S_T = _core_arrays(sch, eids, pos_local, s_edge)
        per_core.append((A_pos, S_T))
    wts = _weights_arrays(np.asarray(W1, np.float32), np.asarray(W2, np.float32),
                          np.asarray(W3, np.float32), np.asarray(b3, np.float32))
    return sch, per_core, wts


def _emulate_core(sch, A_pos, S_T, W1, W2, W3, b3):
    """Numpy emulation of the device program for one core (fp32 math)."""
    R_pad, NB = sch["R_pad"], sch["NB"]
    U = R_pad // SUB
    pos_r = (A_pos.reshape(SUB, POS_DIM, U, BLK).transpose(2, 0, 3, 1)
             .reshape(R_pad, BLK, POS_DIM))
    x1 = pos_r @ W1                                   # [R, BLK, 16]
    t1 = np.maximum(x1, np.minimum(np.exp(x1) - 1, 0))
    x2 = t1 @ W2                                      # [R, BLK, 64]
    v = np.maximum(x2, np.minimum(np.exp(x2) - 1, 0))
    w = v * S_T.T[:, :, None]                         # [R, BLK, 64]
    out = np.zeros((C_OUT, NB * BLK), np.float32)
    for k in range(NB):
        r0, r1 = sch["Roff"][k], sch["Roff"][k + 1]
        if r1 == r0:
            continue
        acc = w[r0:r1].sum(axis=0)                    # [BLK, 64]
        ob = acc @ W3 + b3.reshape(1, -1)             # [BLK, 64]
        out[:, k * BLK:(k + 1) * BLK] = ob.T
    return out


def _unshard(sch, outs, b3):
    """Assemble full [N_OUT, C_OUT] output from per-core [64, NB*BLK]."""
    full = np.broadcast_to(b3.reshape(1, -1), (N_OUT, C_OUT)).astype(np.float32).copy()
    for c in range(N_CORES):
        nodes = sch["nodes"][c]                       # [NB, BLK]
        cols = outs[c].reshape(C_OUT, sch["NB"], BLK)
        mask = (nodes >= 0) & (sch["D"] > 0)[:, None]
        full[nodes[mask]] = cols[:, mask].T
    return full


# ================================================================= device ==
_CELU_OP = None


def _register_celu_op():
    """Register a fused custom DVE op: out = max(in1, min(in0 + imm2, 0)).

    With in0 = exp(x), in1 = x, imm2 = -1 this computes celu(x) in a single
    DVE instruction (celu(x) = max(x, min(e^x - 1, 0))), replacing an ACT/
    GPSIMD pass plus a DVE pass with one DVE pass.
    """
    global _CELU_OP
    if _CELU_OP is not None:
        return _CELU_OP
    import numpy as np
    import concourse.dve_ops as dve_ops
    from concourse.dve_spec import (Spec, Src0, Src1, C2, Zero, lower,
                                    maxx, minn, _has_src1)
    from concourse.dve_uop import DveOpSpec

    name = "CELU_FUSED_PC"
    body = maxx(Src1, minn(Src0 + C2, Zero))
    spec = Spec(
        body=body,
        reference=lambda in0, in1, s0, s1, imm2: np.maximum(
            in1.astype(np.float32),
            np.minimum(in0.astype(np.float32) + imm2, 0.0)).astype(np.float32),
    )
    row = dve_ops._CUSTOM_DVE_ROW_BASE + len(dve_ops.OPS)
    assert row < 0x20
    shas = {}
    for ver in ("v3", "v4"):
        ds = DveOpSpec(name=name, opcode=row, uops=lower(spec, ver=ver),
                       rd1_en=_has_src1(spec))
        shas[ver] = ds.sha(ver)
    op = dve_ops.DveOp(name, spec, subdim=False, uops_sha=shas)
    dve_ops.OPS.append(op)
    dve_ops._SUB_OPCODE_FOR_NAME[name] = row
    dve_ops.CUSTOM_DVE_SPECS[name] = spec
    _CELU_OP = op
    return op


def _build_program(sch, repeat=1):
    import concourse.bass as bass
    import concourse.mybir as mybir
    import concourse.tile as tile
    from concourse import bacc
    from contextlib import ExitStack

    F32 = mybir.dt.float32
    F32R = mybir.dt.float32r
    AF = mybir.ActivationFunctionType
    OP = mybir.AluOpType

    R_pad, NB, D, Roff = sch["R_pad"], sch["NB"], sch["D"], sch["Roff"]
    U = R_pad // SUB
    G = R_pad // RPM

    # round -> block id (or -1)
    blk_of = np.full(R_pad, -1, np.int32)
    for k in range(NB):
        blk_of[Roff[k]:Roff[k + 1]] = k

    celu_op = _register_celu_op()

    nc = bacc.Bacc("TRN2", target_bir_lowering=False, debug=False)

    a_pos = nc.dram_tensor("A_pos", [SUB * POS_DIM, U * BLK], F32R,
                           kind="ExternalInput").ap()
    s_t = nc.dram_tensor("S_T", [BLK, R_pad], F32, kind="ExternalInput").ap()
    w1b = nc.dram_tensor("W1b", [SUB * POS_DIM, BLK], F32R,
                         kind="ExternalInput").ap()
    w2b = nc.dram_tensor("W2b", [BLK, SUB * C_MID], F32R,
                         kind="ExternalInput").ap()
    w3 = nc.dram_tensor("W3", [C_MID, C_OUT], F32R, kind="ExternalInput").ap()
    b3 = nc.dram_tensor("b3", [C_OUT, 1], F32, kind="ExternalInput").ap()
    ident = nc.dram_tensor("ident", [BLK, BLK], F32, kind="ExternalInput").ap()
    out_d = nc.dram_tensor("out", [C_OUT, NB * BLK], F32,
                           kind="ExternalOutput").ap()

    with tile.TileContext(nc) as tc, ExitStack() as ctx:
        const = ctx.enter_context(tc.tile_pool(name="const", bufs=1))
        posp = ctx.enter_context(tc.tile_pool(name="posp", bufs=4))
        p1 = ctx.enter_context(tc.tile_pool(name="p1", bufs=P1_BUFS))
        p2 = ctx.enter_context(tc.tile_pool(name="p2", bufs=P2_BUFS))
        accp = ctx.enter_context(tc.tile_pool(name="accp", bufs=6))
        aggp = ctx.enter_context(tc.tile_pool(name="aggp", bufs=2))
        outp = ctx.enter_context(tc.tile_pool(name="outp", bufs=2))
        ps1 = ctx.enter_context(tc.tile_pool(name="ps1", bufs=PS1_BUFS,
                                             space="PSUM"))
        ps2 = ctx.enter_context(tc.tile_pool(name="ps2", bufs=PS2_BUFS,
                                             space="PSUM"))
        pst = ctx.enter_context(tc.tile_pool(name="pst", bufs=PST_BUFS,
                                             space="PSUM"))
        ps3 = pst if SHARE_PST else ctx.enter_context(
            tc.tile_pool(name="ps3", bufs=1, space="PSUM"))

        # --- constants ----------------------------------------------------
        w1b_sb = const.tile([SUB * POS_DIM, BLK], F32R)
        nc.sync.dma_start(w1b_sb[:], w1b[:])
        w2b_sb = const.tile([BLK, SUB * C_MID], F32R)
        nc.sync.dma_start(w2b_sb[:], w2b[:])
        w3_sb = const.tile([C_MID, C_OUT], F32R)
        nc.sync.dma_start(w3_sb[:], w3[:])
        b3_sb = const.tile([C_OUT, 1], F32)
        nc.sync.dma_start(b3_sb[:], b3[:])
        id_sb = const.tile([BLK, BLK], F32)
        nc.sync.dma_start(id_sb[:], ident[:])
        st_sb = const.tile([BLK, R_pad], F32)
        nc.sync.dma_start(st_sb[:], s_t[:])

        acc_of_blk = {}
        done = []            # finished blocks awaiting mm3

        def flush_mm3():
            nbat = len(done)
            p_t = pst.tile([C_MID, MM3_BATCH * BLK], F32,
                           tag="blkend" if SHARE_PST else "pt")
            for j, (k, acc) in enumerate(done):
                nc.tensor.transpose(p_t[:, j * BLK:(j + 1) * BLK], acc[:],
                                    id_sb[:])
            agg4 = aggp.tile([C_MID, MM3_BATCH * BLK], F32R, tag="agg4")
            nc.scalar.activation(agg4[:, :nbat * BLK], p_t[:, :nbat * BLK],
                                 AF.Copy)
            p3 = ps3.tile([C_OUT, MM3_BATCH * BLK], F32,
                          tag="blkend" if SHARE_PST else "p3")
            nc.tensor.matmul(p3[:, :nbat * BLK], w3_sb[:],
                             agg4[:, :nbat * BLK], start=True, stop=True)
            ot = outp.tile([C_OUT, MM3_BATCH * BLK], F32, tag="ot")
            nc.scalar.activation(ot[:, :nbat * BLK], p3[:, :nbat * BLK],
                                 AF.Identity, bias=b3_sb[:])
            k0 = done[0][0]
            nc.sync.dma_start(out_d[:, k0 * BLK:(k0 + nbat) * BLK],
                              ot[:, :nbat * BLK])
            done.clear()

        for _rep in range(repeat):
          for g in range(G):
            pos_t = posp.tile([SUB * POS_DIM, 4 * BLK], F32R, tag="pos")
            nc.sync.dma_start(pos_t[:], a_pos[:, g * 4 * BLK:(g + 1) * 4 * BLK])

            psum1 = ps1.tile([BLK, 4 * BLK], F32, tag="psum1")
            nc.tensor.matmul(psum1[:], w1b_sb[:], pos_t[:], start=True,
                             stop=True)
            e1 = p1.tile([BLK, 4 * BLK], F32, tag="e1")
            nc.scalar.activation(e1[:], psum1[:], AF.Exp)
            # fused custom DVE op: t1 = max(x, min(e^x - 1, 0)) = celu(x)
            t1 = p1.tile([BLK, 4 * BLK], F32R, tag="t1")
            nc.vector._custom_dve(celu_op, out=t1[:], in0=e1[:],
                                  in1=psum1[:], imm2=-1.0)

            for h in range(RPM // MM2G):
                # two 8-round mm2's land in one [128, 1024] psum tile so the
                # celu2 passes amortize their fixed overheads
                psum2 = ps2.tile([BLK, MM2G * C_MID], F32, tag="psum2")
                for hh in range(MM2G // SUB):
                    u = (MM2G // SUB) * h + hh
                    nc.tensor.matmul(
                        psum2[:, hh * SUB * C_MID:(hh + 1) * SUB * C_MID],
                        t1[:, u * BLK:(u + 1) * BLK],
                        w2b_sb[:], start=True, stop=True)
                e2 = p2.tile([BLK, MM2G * C_MID], F32, tag="e2")
                nc.scalar.activation(e2[:], psum2[:], AF.Exp)
                v = p2.tile([BLK, MM2G * C_MID], F32, tag="v")
                nc.vector._custom_dve(celu_op, out=v[:], in0=e2[:],
                                      in1=psum2[:], imm2=-1.0)

                for r in range(MM2G):
                    R = g * RPM + h * MM2G + r
                    k = blk_of[R]
                    if k < 0:
                        continue
                    vs = v[:, r * C_MID:(r + 1) * C_MID]
                    sc = st_sb[:, R:R + 1]
                    on_pool = (k * POOL_ACC_NUM) % POOL_ACC_DEN < POOL_ACC_NUM
                    if R == Roff[k]:
                        acc = accp.tile([BLK, C_MID], F32, tag="acc")
                        acc_of_blk[k] = acc
                        eng = nc.gpsimd if on_pool else nc.vector
                        eng.tensor_scalar(out=acc[:], in0=vs, scalar1=sc,
                                          scalar2=0.0, op0=OP.mult, op1=OP.add)
                    elif on_pool:
                        # walrus rejects scalar_tensor_tensor on Pool; use a
                        # scale-into-temp + add pair instead
                        acc = acc_of_blk[k]
                        w = accp.tile([BLK, C_MID], F32, tag="accw")
                        nc.gpsimd.tensor_scalar(out=w[:], in0=vs, scalar1=sc,
                                                scalar2=0.0, op0=OP.mult,
                                                op1=OP.add)
                        nc.gpsimd.tensor_add(acc[:], acc[:], w[:])
                    else:
                        acc = acc_of_blk[k]
                        nc.vector.scalar_tensor_tensor(out=acc[:], in0=vs,
                                                       scalar=sc, in1=acc[:],
                                                       op0=OP.mult, op1=OP.add)
                    if R == Roff[k] + D[k] - 1:
                        done.append((k, acc_of_blk.pop(k)))
                        if len(done) == MM3_BATCH:
                            flush_mm3()
          if done:
            flush_mm3()

    nc.compile()
    return nc


# ================================================================= runner ==
class _Runner:
    """Cached shard_map-jit wrapper around the bass_exec custom call.

    Mirrors concourse.bass2jax.run_bass_via_pjrt's multi-core branch, but
    keeps the jitted callable (and optionally device-resident inputs) so the
    kernel can be re-executed without re-tracing/re-compiling.
    """

    def __init__(self, nc):
        import jax
        import numpy as np
        from jax.sharding import Mesh, PartitionSpec, NamedSharding
        from jax.experimental.shard_map import shard_map
        import concourse.mybir as mybir
        from concourse.bass2jax import (_bass_exec_p, install_neuronx_cc_hook,
                                        partition_id_tensor)

        install_neuronx_cc_hook()
        self.nc = nc
        part_name = (nc.partition_id_tensor.name
                     if nc.partition_id_tensor is not None else None)
        in_names, out_names, out_avals, zero_outs = [], [], [], []
        for alloc in nc.m.functions[0].allocations:
            if not isinstance(alloc, mybir.MemoryLocationSet):
                continue
            name = alloc.memorylocations[0].name
            if alloc.kind == "ExternalInput":
                if name != part_name:
                    in_names.append(name)
            elif alloc.kind == "ExternalOutput":
                shape = tuple(alloc.tensor_shape)
                dtype = mybir.dt.np(alloc.dtype)
                out_names.append(name)
                out_avals.append(jax.core.ShapedArray(shape, dtype))
                zero_outs.append(np.zeros(shape, dtype))
        self.in_names = list(in_names)
        self.out_names = out_names
        self.out_avals = out_avals
        self.zero_outs = zero_outs
        all_in_names = list(in_names) + list(out_names)
        if part_name is not None:
            all_in_names.append(part_name)
        all_in_names = tuple(all_in_names)

        def _body(*args):
            operands = list(args)
            if part_name is not None:
                operands.append(partition_id_tensor())
            outs = _bass_exec_p.bind(
                *operands, out_avals=tuple(out_avals), in_names=all_in_names,
                out_names=tuple(out_names), lowering_input_output_aliases=(),
                sim_require_finite=True, sim_require_nnan=True, nc=nc)
            return tuple(outs)

        devices = jax.devices()[:N_CORES]
        self.mesh = Mesh(np.asarray(devices), ("core",))
        n_args = len(in_names) + len(zero_outs)
        in_specs = (PartitionSpec("core"),) * n_args
        out_specs = (PartitionSpec("core"),) * len(out_names)
        self.sharding = NamedSharding(self.mesh, PartitionSpec("core"))
        self.fn = jax.jit(shard_map(_body, mesh=self.mesh, in_specs=in_specs,
                                    out_specs=out_specs, check_rep=False),
                          keep_unused=True)

    def stage(self, in_maps):
        """Concatenate per-core inputs and place them on the devices."""
        import jax
        import numpy as np
        concat = [np.concatenate([np.asarray(m[n]) for m in in_maps], axis=0)
                  for n in self.in_names]
        concat += [np.concatenate([z] * N_CORES, axis=0)
                   for z in self.zero_outs]
        return [jax.device_put(a, self.sharding) for a in concat]

    def run(self, staged):
        import jax
        outs = self.fn(*staged)
        jax.block_until_ready(outs)
        return outs

    def results(self, outs):
        import numpy as np
        per_core = []
        for c in range(N_CORES):
            per_core.append({
                n: np.asarray(outs[i]).reshape(N_CORES, *self.out_avals[i].shape)[c]
                for i, n in enumerate(self.out_names)})
        return per_core


_RUNNER = None
_LAST = {}


def _run_on_device(nc, in_maps):
    global _RUNNER
    if _RUNNER is None or _RUNNER.nc is not nc:
        _RUNNER = _Runner(nc)
    staged = _RUNNER.stage(in_maps)
    outs = _RUNNER.run(staged)
    _LAST["runner"] = _RUNNER
    _LAST["staged"] = staged
    return [r["out"] for r in _RUNNER.results(outs)]


def kernel(x_in, pos_in, batch_in, pos_out, in_index, out_index,
           W1, W2, W3, b3):
    sch, per_core, wts = _prepare(x_in, pos_in, pos_out, in_index, out_index,
                                  W1, W2, W3, b3)
    W1b, W2b, W3a, b3a, ident = wts

    key = (sch["R_pad"], sch["NB"], tuple(sch["D"].tolist()))
    if key not in _COMPILED:
        _COMPILED.clear()
        _COMPILED[key] = _build_program(sch)
    nc = _COMPILED[key]

    in_maps = []
    for c in range(N_CORES):
        A_pos, S_T = per_core[c]
        in_maps.append({"A_pos": A_pos, "S_T": S_T, "W1b": W1b, "W2b": W2b,
                        "W3": W3a, "b3": b3a, "ident": ident})
    outs = _run_on_device(nc, in_maps)
    return _unshard(sch, outs, b3a)


# revision 37
# speedup vs baseline: 14394.7265x; 49.4721x over previous
"""PointConv GNN message passing on 8 Trainium2 NeuronCores.

Strategy
--------
out_index is sorted, C_IN == 1, avg degree 16. The host reorders edges into a
dense (block, round, lane) grid: nodes are sorted by degree and grouped into
128-node blocks; round r of block b processes edge r of each of the block's
128 nodes (lane = node). Each node's edge list is padded to the block max
degree (~6% padding, zero-scale so pads contribute nothing).

This turns both gathers and the segment-sum into dense streaming work:
  - per-edge inputs (relative positions, scales) are staged by the host in
    exactly the SBUF layout the device consumes,
  - the segment-sum becomes a per-round accumulate into a [128 nodes, 64]
    SBUF accumulator where the scale is a per-partition scalar.

Device pipeline per 16-round macro (2048 edge slots):
  mm1: blockdiag(W1 x8) [24,128] @ posT [24,256] -> psum1 [128,256]   (f32r)
  celu1 = max(x, min(exp(x)-1, 0)):  Exp on ACT, min on GPSIMD, max on DVE
  mm2 (x2): t1 [128,128] @ blockdiag(W2 x8) [128,512] -> psum2 [128,512]
  celu2 likewise; then per round: acc = (v_r * s_r) + acc  (DVE stt)
Block end: PE-transpose acc, mm3 with W3 (4 blocks batched), +b3 on ACT, DMA.

Nodes are partitioned across cores (whole blocks dealt round-robin), so there
are no cross-core reductions at all. All 8 cores run one identical program.
"""

import numpy as np

# ---------------------------------------------------------------- constants
N_IN = 100000
N_OUT = 100000
E = 1600000
C_MID = 64
C_OUT = 64
POS_DIM = 3
N_CORES = 8
BLK = 128          # nodes per block (= lanes)
RPM = 32           # rounds per mm1 super-macro (pos tile)
SUB = 8            # rounds per mm2 matmul (blockdiag W2 x8)
MM2G = 16          # rounds per celu2 tile ([128, 1024] psum)
MM3_BATCH = 4      # blocks per mm3

_COMPILED = {}     # (structure key) -> (nc, sharded_fn, meta)
POOL_ACC_NUM, POOL_ACC_DEN = 5, 17  # fraction of blocks whose acc chain runs
                                     # on GPSIMD instead of DVE
PS1_BUFS, PS2_BUFS = 2, 2           # PSUM double-buffer depths (banks are
                                    # scarce: ps1*1 + ps2*2 + 2 <= 8)
PST_BUFS = 1
SHARE_PST = False                   # share one PSUM pool between transpose+mm3
P1_BUFS, P2_BUFS = 4, 6


# =================================================================== host ==
def _schedule(deg):
    """Build the shared block/round structure and per-core node lists.

    Returns dict with:
      nodes   : [N_CORES, NB, BLK] int32 node ids (-1 pad)
      D       : [NB] int32 rounds per block (shared across cores)
      Roff    : [NB] int32 round offset of each block
      R_pad   : int, total rounds padded to RPM multiple
    """
    order = np.argsort(-deg, kind="stable").astype(np.int32)
    nbg = -(-N_OUT // BLK)                       # global blocks
    pad = nbg * BLK - N_OUT
    order_p = np.concatenate([order, np.full(pad, -1, np.int32)])
    gblocks = order_p.reshape(nbg, BLK)          # global block -> node ids
    gdeg = np.where(gblocks >= 0, deg[np.maximum(gblocks, 0)], 0)
    gmax = gdeg.max(axis=1)                      # per global block max degree

    NB = -(-nbg // N_CORES)
    nodes = np.full((N_CORES, NB, BLK), -1, np.int32)
    D = np.zeros(NB, np.int32)
    for k in range(NB):
        for c in range(N_CORES):
            g = k * N_CORES + c
            if g < nbg:
                nodes[c, k] = gblocks[g]
                D[k] = max(D[k], gmax[g])
    Roff = np.concatenate([[0], np.cumsum(D)]).astype(np.int32)
    R = int(Roff[-1])
    R_pad = -(-R // RPM) * RPM
    return dict(nodes=nodes, D=D, Roff=Roff, R=R, R_pad=R_pad, NB=NB)


def _edge_table(sch, deg, start, core):
    """[R_pad, BLK] int32 edge ids for one core (-1 = pad slot)."""
    nodes = sch["nodes"][core]          # [NB, BLK]
    D, Roff, R_pad = sch["D"], sch["Roff"], sch["R_pad"]
    R = sch["R"]
    blk_of_row = np.repeat(np.arange(len(D), dtype=np.int32), D)    # [R]
    r_in_blk = np.arange(R, dtype=np.int32) - Roff[blk_of_row]
    nd = nodes[blk_of_row]                                          # [R, BLK]
    dg = np.where(nd >= 0, deg[np.maximum(nd, 0)], 0)
    st = np.where(nd >= 0, start[np.maximum(nd, 0)], 0)
    eid = st + r_in_blk[:, None]
    valid = (nd >= 0) & (r_in_blk[:, None] < dg)
    eid = np.where(valid, eid, -1).astype(np.int32)
    out = np.full((R_pad, BLK), -1, np.int32)
    out[:R] = eid
    return out


def _core_arrays(sch, eids, pos_local, s_edge):
    """Device input arrays for one core."""
    R_pad = sch["R_pad"]
    U = R_pad // SUB
    idx = np.where(eids >= 0, eids, E)               # E -> zero pad row
    plp = np.concatenate([pos_local, np.zeros((1, POS_DIM), np.float32)])
    sp = np.concatenate([s_edge, np.zeros(1, np.float32)])
    pos_r = plp[idx]                                  # [R_pad, BLK, 3]
    # A_pos[3*rr + a, 128*u + lane] = pos_r[8u + rr, lane, a]
    A_pos = np.ascontiguousarray(
        pos_r.reshape(U, SUB, BLK, POS_DIM).transpose(1, 3, 0, 2)
        .reshape(SUB * POS_DIM, U * BLK))
    S_T = np.ascontiguousarray(sp[idx].T)             # [BLK, R_pad]
    return A_pos, S_T


def _weights_arrays(W1, W2, W3, b3):
    W1b = np.zeros((SUB * POS_DIM, BLK), np.float32)
    for r in range(SUB):
        W1b[3 * r:3 * r + 3, 16 * r:16 * r + 16] = W1
    W2b = np.zeros((BLK, SUB * C_MID), np.float32)
    for r in range(SUB):
        W2b[16 * r:16 * r + 16, 64 * r:64 * r + 64] = W2
    ident = np.eye(BLK, dtype=np.float32)
    return W1b, W2b, np.ascontiguousarray(W3.astype(np.float32)), \
        b3.reshape(C_OUT, 1).astype(np.float32), ident


def _prepare(x_in, pos_in, pos_out, in_index, out_index, W1, W2, W3, b3):
    i32 = np.asarray(in_index, np.int64).astype(np.int32)
    o32 = np.asarray(out_index, np.int64).astype(np.int32)
    x = np.asarray(x_in, np.float32).reshape(-1)
    deg = np.bincount(o32, minlength=N_OUT).astype(np.int32)
    start = np.concatenate([[0], np.cumsum(deg)[:-1]]).astype(np.int64)
    degc = np.maximum(deg, 1).astype(np.float32)
    s_edge = (x[i32] / degc[o32]).astype(np.float32)
    pos_local = (np.asarray(pos_in, np.float32)[i32]
                 - np.asarray(pos_out, np.float32)[o32])
    sch = _schedule(deg)
    per_core = []
    for c in range(N_CORES):
        eids = _edge_table(sch, deg, start, c)
        A_pos, S_T = _core_arrays(sch, eids, pos_local, s_edge)
        per_core.append((A_pos, S_T))
    wts = _weights_arrays(np.asarray(W1, np.float32), np.asarray(W2, np.float32),
                          np.asarray(W3, np.float32), np.asarray(b3, np.float32))
    return sch, per_core, wts


def _emulate_core(sch, A_pos, S_T, W1, W2, W3, b3):
    """Numpy emulation of the device program for one core (fp32 math)."""
    R_pad, NB = sch["R_pad"], sch["NB"]
    U = R_pad // SUB
    pos_r = (A_pos.reshape(SUB, POS_DIM, U, BLK).transpose(2, 0, 3, 1)
             .reshape(R_pad, BLK, POS_DIM))
    x1 = pos_r @ W1                                   # [R, BLK, 16]
    t1 = np.maximum(x1, np.minimum(np.exp(x1) - 1, 0))
    x2 = t1 @ W2                                      # [R, BLK, 64]
    v = np.maximum(x2, np.minimum(np.exp(x2) - 1, 0))
    w = v * S_T.T[:, :, None]                         # [R, BLK, 64]
    out = np.zeros((C_OUT, NB * BLK), np.float32)
    for k in range(NB):
        r0, r1 = sch["Roff"][k], sch["Roff"][k + 1]
        if r1 == r0:
            continue
        acc = w[r0:r1].sum(axis=0)                    # [BLK, 64]
        ob = acc @ W3 + b3.reshape(1, -1)             # [BLK, 64]
        out[:, k * BLK:(k + 1) * BLK] = ob.T
    return out


def _unshard(sch, outs, b3):
    """Assemble full [N_OUT, C_OUT] output from per-core [64, NB*BLK]."""
    full = np.broadcast_to(b3.reshape(1, -1), (N_OUT, C_OUT)).astype(np.float32).copy()
    for c in range(N_CORES):
        nodes = sch["nodes"][c]                       # [NB, BLK]
        cols = outs[c].reshape(C_OUT, sch["NB"], BLK)
        mask = (nodes >= 0) & (sch["D"] > 0)[:, None]
        full[nodes[mask]] = cols[:, mask].T
    return full


# ================================================================= device ==
_CELU_OP = None


def _register_celu_op():
    """Register a fused custom DVE op: out = max(in1, min(in0 + imm2, 0)).

    With in0 = exp(x), in1 = x, imm2 = -1 this computes celu(x) in a single
    DVE instruction (celu(x) = max(x, min(e^x - 1, 0))), replacing an ACT/
    GPSIMD pass plus a DVE pass with one DVE pass.
    """
    global _CELU_OP
    if _CELU_OP is not None:
        return _CELU_OP
    import numpy as np
    import concourse.dve_ops as dve_ops
    from concourse.dve_spec import (Spec, Src0, Src1, C2, Zero, lower,
                                    maxx, minn, _has_src1)
    from concourse.dve_uop import DveOpSpec

    name = "CELU_FUSED_PC"
    body = maxx(Src1, minn(Src0 + C2, Zero))
    spec = Spec(
        body=body,
        reference=lambda in0, in1, s0, s1, imm2: np.maximum(
            in1.astype(np.float32),
            np.minimum(in0.astype(np.float32) + imm2, 0.0)).astype(np.float32),
    )
    row = dve_ops._CUSTOM_DVE_ROW_BASE + len(dve_ops.OPS)
    assert row < 0x20
    shas = {}
    for ver in ("v3", "v4"):
        ds = DveOpSpec(name=name, opcode=row, uops=lower(spec, ver=ver),
                       rd1_en=_has_src1(spec))
        shas[ver] = ds.sha(ver)
    op = dve_ops.DveOp(name, spec, subdim=False, uops_sha=shas)
    dve_ops.OPS.append(op)
    dve_ops._SUB_OPCODE_FOR_NAME[name] = row
    dve_ops.CUSTOM_DVE_SPECS[name] = spec
    _CELU_OP = op
    return op


def _build_program(sch, repeat=1):
    import concourse.bass as bass
    import concourse.mybir as mybir
    import concourse.tile as tile
    from concourse import bacc
    from contextlib import ExitStack

    F32 = mybir.dt.float32
    F32R = mybir.dt.float32r
    AF = mybir.ActivationFunctionType
    OP = mybir.AluOpType

    R_pad, NB, D, Roff = sch["R_pad"], sch["NB"], sch["D"], sch["Roff"]
    U = R_pad // SUB
    G = R_pad // RPM

    # round -> block id (or -1)
    blk_of = np.full(R_pad, -1, np.int32)
    for k in range(NB):
        blk_of[Roff[k]:Roff[k + 1]] = k

    celu_op = _register_celu_op()

    nc = bacc.Bacc("TRN2", target_bir_lowering=False, debug=False)

    a_pos = nc.dram_tensor("A_pos", [SUB * POS_DIM, U * BLK], F32R,
                           kind="ExternalInput").ap()
    s_t = nc.dram_tensor("S_T", [BLK, R_pad], F32, kind="ExternalInput").ap()
    w1b = nc.dram_tensor("W1b", [SUB * POS_DIM, BLK], F32R,
                         kind="ExternalInput").ap()
    w2b = nc.dram_tensor("W2b", [BLK, SUB * C_MID], F32R,
                         kind="ExternalInput").ap()
    w3 = nc.dram_tensor("W3", [C_MID, C_OUT], F32R, kind="ExternalInput").ap()
    b3 = nc.dram_tensor("b3", [C_OUT, 1], F32, kind="ExternalInput").ap()
    ident = nc.dram_tensor("ident", [BLK, BLK], F32, kind="ExternalInput").ap()
    out_d = nc.dram_tensor("out", [C_OUT, NB * BLK], F32,
                           kind="ExternalOutput").ap()

    with tile.TileContext(nc) as tc, ExitStack() as ctx:
        const = ctx.enter_context(tc.tile_pool(name="const", bufs=1))
        posp = ctx.enter_context(tc.tile_pool(name="posp", bufs=4))
        p1 = ctx.enter_context(tc.tile_pool(name="p1", bufs=P1_BUFS))
        p2 = ctx.enter_context(tc.tile_pool(name="p2", bufs=P2_BUFS))
        accp = ctx.enter_context(tc.tile_pool(name="accp", bufs=6))
        aggp = ctx.enter_context(tc.tile_pool(name="aggp", bufs=2))
        outp = ctx.enter_context(tc.tile_pool(name="outp", bufs=2))
        ps1 = ctx.enter_context(tc.tile_pool(name="ps1", bufs=PS1_BUFS,
                                             space="PSUM"))
        ps2 = ctx.enter_context(tc.tile_pool(name="ps2", bufs=PS2_BUFS,
                                             space="PSUM"))
        pst = ctx.enter_context(tc.tile_pool(name="pst", bufs=PST_BUFS,
                                             space="PSUM"))
        ps3 = pst if SHARE_PST else ctx.enter_context(
            tc.tile_pool(name="ps3", bufs=1, space="PSUM"))

        # --- constants ----------------------------------------------------
        w1b_sb = const.tile([SUB * POS_DIM, BLK], F32R)
        nc.sync.dma_start(w1b_sb[:], w1b[:])
        w2b_sb = const.tile([BLK, SUB * C_MID], F32R)
        nc.sync.dma_start(w2b_sb[:], w2b[:])
        w3_sb = const.tile([C_MID, C_OUT], F32R)
        nc.sync.dma_start(w3_sb[:], w3[:])
        b3_sb = const.tile([C_OUT, 1], F32)
        nc.sync.dma_start(b3_sb[:], b3[:])
        id_sb = const.tile([BLK, BLK], F32)
        nc.sync.dma_start(id_sb[:], ident[:])
        st_sb = const.tile([BLK, R_pad], F32)
        nc.sync.dma_start(st_sb[:], s_t[:])

        acc_of_blk = {}
        done = []            # finished blocks awaiting mm3

        def flush_mm3():
            nbat = len(done)
            p_t = pst.tile([C_MID, MM3_BATCH * BLK], F32,
                           tag="blkend" if SHARE_PST else "pt")
            for j, (k, acc) in enumerate(done):
                nc.tensor.transpose(p_t[:, j * BLK:(j + 1) * BLK], acc[:],
                                    id_sb[:])
            agg4 = aggp.tile([C_MID, MM3_BATCH * BLK], F32R, tag="agg4")
            nc.scalar.activation(agg4[:, :nbat * BLK], p_t[:, :nbat * BLK],
                                 AF.Copy)
            p3 = ps3.tile([C_OUT, MM3_BATCH * BLK], F32,
                          tag="blkend" if SHARE_PST else "p3")
            nc.tensor.matmul(p3[:, :nbat * BLK], w3_sb[:],
                             agg4[:, :nbat * BLK], start=True, stop=True)
            ot = outp.tile([C_OUT, MM3_BATCH * BLK], F32, tag="ot")
            nc.scalar.activation(ot[:, :nbat * BLK], p3[:, :nbat * BLK],
                                 AF.Identity, bias=b3_sb[:])
            k0 = done[0][0]
            nc.sync.dma_start(out_d[:, k0 * BLK:(k0 + nbat) * BLK],
                              ot[:, :nbat * BLK])
            done.clear()

        for _rep in range(repeat):
          for g in range(G):
            pos_t = posp.tile([SUB * POS_DIM, 4 * BLK], F32R, tag="pos")
            nc.sync.dma_start(pos_t[:], a_pos[:, g * 4 * BLK:(g + 1) * 4 * BLK])

            psum1 = ps1.tile([BLK, 4 * BLK], F32, tag="psum1")
            nc.tensor.matmul(psum1[:], w1b_sb[:], pos_t[:], start=True,
                             stop=True)
            e1 = p1.tile([BLK, 4 * BLK], F32, tag="e1")
            nc.scalar.activation(e1[:], psum1[:], AF.Exp)
            # fused custom DVE op: t1 = max(x, min(e^x - 1, 0)) = celu(x)
            t1 = p1.tile([BLK, 4 * BLK], F32R, tag="t1")
            nc.vector._custom_dve(celu_op, out=t1[:], in0=e1[:],
                                  in1=psum1[:], imm2=-1.0)

            for h in range(RPM // MM2G):
                # two 8-round mm2's land in one [128, 1024] psum tile so the
                # celu2 passes amortize their fixed overheads
                psum2 = ps2.tile([BLK, MM2G * C_MID], F32, tag="psum2")
                for hh in range(MM2G // SUB):
                    u = (MM2G // SUB) * h + hh
                    nc.tensor.matmul(
                        psum2[:, hh * SUB * C_MID:(hh + 1) * SUB * C_MID],
                        t1[:, u * BLK:(u + 1) * BLK],
                        w2b_sb[:], start=True, stop=True)
                e2 = p2.tile([BLK, MM2G * C_MID], F32, tag="e2")
                nc.scalar.activation(e2[:], psum2[:], AF.Exp)
                v = p2.tile([BLK, MM2G * C_MID], F32, tag="v")
                nc.vector._custom_dve(celu_op, out=v[:], in0=e2[:],
                                      in1=psum2[:], imm2=-1.0)

                for r in range(MM2G):
                    R = g * RPM + h * MM2G + r
                    k = blk_of[R]
                    if k < 0:
                        continue
                    vs = v[:, r * C_MID:(r + 1) * C_MID]
                    sc = st_sb[:, R:R + 1]
                    on_pool = (k * POOL_ACC_NUM) % POOL_ACC_DEN < POOL_ACC_NUM
                    if R == Roff[k]:
                        acc = accp.tile([BLK, C_MID], F32, tag="acc")
                        acc_of_blk[k] = acc
                        eng = nc.gpsimd if on_pool else nc.vector
                        eng.tensor_scalar(out=acc[:], in0=vs, scalar1=sc,
                                          scalar2=0.0, op0=OP.mult, op1=OP.add)
                    elif on_pool:
                        # walrus rejects scalar_tensor_tensor on Pool; use a
                        # scale-into-temp + add pair instead
                        acc = acc_of_blk[k]
                        w = accp.tile([BLK, C_MID], F32, tag="accw")
                        nc.gpsimd.tensor_scalar(out=w[:], in0=vs, scalar1=sc,
                                                scalar2=0.0, op0=OP.mult,
                                                op1=OP.add)
                        nc.gpsimd.tensor_add(acc[:], acc[:], w[:])
                    else:
                        acc = acc_of_blk[k]
                        nc.vector.scalar_tensor_tensor(out=acc[:], in0=vs,
                                                       scalar=sc, in1=acc[:],
                                                       op0=OP.mult, op1=OP.add)
                    if R == Roff[k] + D[k] - 1:
                        done.append((k, acc_of_blk.pop(k)))
                        if len(done) == MM3_BATCH:
                            flush_mm3()
          if done:
            flush_mm3()

    nc.compile()
    return nc


# ================================================================= runner ==
class _Runner:
    """Cached shard_map-jit wrapper around the bass_exec custom call.

    Mirrors concourse.bass2jax.run_bass_via_pjrt's multi-core branch, but
    keeps the jitted callable (and optionally device-resident inputs) so the
    kernel can be re-executed without re-tracing/re-compiling.
    """

    def __init__(self, nc):
        import jax
        import numpy as np
        from jax.sharding import Mesh, PartitionSpec, NamedSharding
        from jax.experimental.shard_map import shard_map
        import concourse.mybir as mybir
        from concourse.bass2jax import (_bass_exec_p, install_neuronx_cc_hook,
                                        partition_id_tensor)

        install_neuronx_cc_hook()
        self.nc = nc
        part_name = (nc.partition_id_tensor.name
                     if nc.partition_id_tensor is not None else None)
        in_names, out_names, out_avals, zero_outs = [], [], [], []
        for alloc in nc.m.functions[0].allocations:
            if not isinstance(alloc, mybir.MemoryLocationSet):
                continue
            name = alloc.memorylocations[0].name
            if alloc.kind == "ExternalInput":
                if name != part_name:
                    in_names.append(name)
            elif alloc.kind == "ExternalOutput":
                shape = tuple(alloc.tensor_shape)
                dtype = mybir.dt.np(alloc.dtype)
                out_names.append(name)
                out_avals.append(jax.core.ShapedArray(shape, dtype))
                zero_outs.append(np.zeros(shape, dtype))
        self.in_names = list(in_names)
        self.out_names = out_names
        self.out_avals = out_avals
        self.zero_outs = zero_outs
        all_in_names = list(in_names) + list(out_names)
        if part_name is not None:
            all_in_names.append(part_name)
        all_in_names = tuple(all_in_names)

        def _body(*args):
            operands = list(args)
            if part_name is not None:
                operands.append(partition_id_tensor())
            outs = _bass_exec_p.bind(
                *operands, out_avals=tuple(out_avals), in_names=all_in_names,
                out_names=tuple(out_names), lowering_input_output_aliases=(),
                sim_require_finite=True, sim_require_nnan=True, nc=nc)
            return tuple(outs)

        devices = jax.devices()[:N_CORES]
        self.mesh = Mesh(np.asarray(devices), ("core",))
        n_args = len(in_names) + len(zero_outs)
        in_specs = (PartitionSpec("core"),) * n_args
        out_specs = (PartitionSpec("core"),) * len(out_names)
        self.sharding = NamedSharding(self.mesh, PartitionSpec("core"))
        self.fn = jax.jit(shard_map(_body, mesh=self.mesh, in_specs=in_specs,
                                    out_specs=out_specs, check_rep=False),
                          keep_unused=True)

    def stage(self, in_maps):
        """Concatenate per-core inputs and place them on the devices."""
        import jax
        import numpy as np
        concat = [np.concatenate([np.asarray(m[n]) for m in in_maps], axis=0)
                  for n in self.in_names]
        concat += [np.concatenate([z] * N_CORES, axis=0)
                   for z in self.zero_outs]
        return [jax.device_put(a, self.sharding) for a in concat]

    def run(self, staged):
        import jax
        outs = self.fn(*staged)
        jax.block_until_ready(outs)
        return outs

    def results(self, outs):
        import numpy as np
        per_core = []
        for c in range(N_CORES):
            per_core.append({
                n: np.asarray(outs[i]).reshape(N_CORES, *self.out_avals[i].shape)[c]
                for i, n in enumerate(self.out_names)})
        return per_core


_RUNNER = None
_LAST = {}


def _run_on_device(nc, in_maps):
    global _RUNNER
    if _RUNNER is None or _RUNNER.nc is not nc:
        _RUNNER = _Runner(nc)
    staged = _RUNNER.stage(in_maps)
    outs = _RUNNER.run(staged)
    _LAST["runner"] = _RUNNER
    _LAST["staged"] = staged
    return [r["out"] for r in _RUNNER.results(outs)]


def kernel(x_in, pos_in, batch_in, pos_out, in_index, out_index,
           W1, W2, W3, b3):
    sch, per_core, wts = _prepare(x_in, pos_in, pos_out, in_index, out_index,
                                  W1, W2, W3, b3)
    W1b, W2b, W3a, b3a, ident = wts

    key = (sch["R_pad"], sch["NB"], tuple(sch["D"].tolist()))
    if key not in _COMPILED:
        _COMPILED.clear()
        _COMPILED[key] = _build_program(sch)
    nc = _COMPILED[key]

    in_maps = []
    for c in range(N_CORES):
        A_pos, S_T = per_core[c]
        in_maps.append({"A_pos": A_pos, "S_T": S_T, "W1b": W1b, "W2b": W2b,
                        "W3": W3a, "b3": b3a, "ident": ident})
    outs = _run_on_device(nc, in_maps)
    return _unshard(sch, outs, b3a)
